# revision 14
# baseline (speedup 1.0000x reference)
"""Grok-1 MoE kernel for 8 Trainium2 NeuronCores.

Strategy (expert-parallel, host-side routing):
  - Host: gating in fp64 (logits -> softcap tanh -> softmax -> top-2),
    build per-expert token lists, gather tokens into a fixed-capacity
    buffer (C=640 >= max expert load for T=2048, top2 of 8 experts),
    pre-transpose/pre-tile all operands so every device DMA is contiguous.
  - Device (core e = expert e): aT = w1eT^T @ xT, bT = w3eT^T @ xT,
    hT = gelu(aT) * bT  (bf16), y = hT^T @ w2eT scaled per-token by the
    combine weight.  All matmuls in bf16, accumulate fp32 in PSUM.
  - Host: scatter-add the per-expert outputs into the full [T, D] output.

Walrus codegen constraint: dynamic DMA instructions accept only ONE sync
wait; DVE TensorTensor likewise.  Hence: <=16 DMAs per queue (so ring
waits never fire) and both tensor_mul operands produced by ACT (one sem).
"""

import sys

sys.path.insert(0, "/opt/trn_rl_repo")

import numpy as np

P = 128
T = 2048
D = 2048
I = 2048
E = 8
C = 640  # per-expert token capacity (max observed load 554; mean 512)
ND = D // P  # 16 d-tiles
NI = I // P  # 16 i-tiles
TCH = 320  # phase-A psum column chunk (320 * 4B = 1280B < 2KB bank)
DDC = 512  # phase-B output column chunk (one psum bank)
NDD = D // DDC

_cache = {}


def _build(cap, reps=1):
    from concourse import bass, tile, mybir

    tt = cap // P
    nch = cap // TCH
    bf16 = mybir.dt.bfloat16
    f32 = mybir.dt.float32

    nc = bass.Bass()
    x_d = nc.dram_tensor("xc", [P, ND, cap], bf16, kind="ExternalInput")
    w13_d = nc.dram_tensor("w13c", [NI, P, 2, ND, P], bf16, kind="ExternalInput")
    w2_d = nc.dram_tensor("w2c", [P, NI, D], bf16, kind="ExternalInput")
    gw_d = nc.dram_tensor("gwc", [P, tt], f32, kind="ExternalInput")
    y_d = nc.dram_tensor("y", [tt, P, NDD, DDC], f32, kind="ExternalOutput")

    Gelu = mybir.ActivationFunctionType.Gelu
    Copy = mybir.ActivationFunctionType.Copy

    with tile.TileContext(nc) as tc:
        with (
            tc.tile_pool(name="xp", bufs=1) as xp,
            tc.tile_pool(name="wp", bufs=3) as wp,
            tc.tile_pool(name="w2p", bufs=1) as w2p,
            tc.tile_pool(name="hp", bufs=1) as hp,
            tc.tile_pool(name="gp", bufs=1) as gp,
            tc.tile_pool(name="ab", bufs=4) as ab,
            tc.tile_pool(name="yp", bufs=5) as yp,
            tc.tile_pool(name="ps", bufs=2, space="PSUM") as ps,
        ):
            xs = xp.tile([P, ND, cap], bf16)
            nc.scalar.dma_start(out=xs[:], in_=x_d[:])
            gs = gp.tile([P, tt], f32)
            nc.scalar.dma_start(out=gs[:], in_=gw_d[:])
            w2s = w2p.tile([P, NI, D], bf16)
            nc.scalar.dma_start(out=w2s[:], in_=w2_d[:])
            hs = hp.tile([P, NI, cap], bf16)

            for _rep in range(reps):
                _phases(nc, tc, wp, ab, yp, ps, xs, gs, w2s, hs,
                        w13_d, y_d, cap, tt, nch, Gelu, Copy)

    return nc


def _phases(nc, tc, wp, ab, yp, ps, xs, gs, w2s, hs, w13_d, y_d,
            cap, tt, nch, Gelu, Copy):
    from concourse import mybir
    bf16 = mybir.dt.bfloat16
    f32 = mybir.dt.float32
    if True:
        if True:
            # Phase A: hT[i, t] = gelu(aT) * bT  for i-tile blocks
            for it in range(NI):
                w13b = wp.tile([P, 2, ND, P], bf16, tag="wb")
                nc.sync.dma_start(out=w13b[:], in_=w13_d[it])
                for ch in range(nch):
                    t0 = ch * TCH
                    pa = ps.tile([P, TCH], f32, tag="pa")
                    pb = ps.tile([P, TCH], f32, tag="pb")
                    for dt in range(ND):
                        nc.tensor.matmul(
                            pa[:],
                            w13b[:, 0, dt, :],
                            xs[:, dt, t0 : t0 + TCH],
                            start=(dt == 0),
                            stop=(dt == ND - 1),
                        )
                    for dt in range(ND):
                        nc.tensor.matmul(
                            pb[:],
                            w13b[:, 1, dt, :],
                            xs[:, dt, t0 : t0 + TCH],
                            start=(dt == 0),
                            stop=(dt == ND - 1),
                        )
                    ga = ab.tile([P, TCH], f32, tag="ga")
                    nc.scalar.activation(ga[:], pa[:], Gelu)
                    bs = ab.tile([P, TCH], f32, tag="bs")
                    nc.scalar.activation(bs[:], pb[:], Copy)
                    nc.vector.tensor_mul(hs[:, it, t0 : t0 + TCH], ga[:], bs[:])

            # Phase B: y[t, d] = sum_i hT[i, t] * w2T[i, d], scaled by gw[t]
            for ti in range(tt):
                yo = yp.tile([P, NDD, DDC], f32, tag="yo")
                for dd in range(NDD):
                    py = ps.tile([P, DDC], f32, tag="py")
                    for it in range(NI):
                        nc.tensor.matmul(
                            py[:],
                            hs[:, it, ti * P : (ti + 1) * P],
                            w2s[:, it, dd * DDC : (dd + 1) * DDC],
                            start=(it == 0),
                            stop=(it == NI - 1),
                        )
                    nc.scalar.activation(
                        yo[:, dd, :], py[:], Copy, scale=gs[:, ti : ti + 1]
                    )
                nc.scalar.dma_start(out=y_d[ti], in_=yo[:])

    return nc


_WAIT_LIMITS = {"Matmult": 2}
_WAIT_SKIP = {
    "EventSemaphore",
    "UnconditionalBranch",
    "ConditionalBranch",
    "RegisterMove",
    "Call",
    "ISA",
}


def _legalize_waits(ant_bir_str):
    """Walrus codegen allows only 1 sync-wait on most instruction structs
    (2 on Matmult).  Tile can emit more; hoist the excess onto standalone
    EventSemaphore (pure wait) instructions inserted just before, on the
    same engine stream."""
    import orjson

    d = orjson.loads(ant_bir_str)
    n_fix = 0
    for fn in d.get("functions", []):
        for blk in fn.get("blocks", []):
            out = []
            for inst in blk.get("instructions", []):
                si = inst.get("sync_info") or {}
                waits = si.get("on_wait") or []
                op = inst.get("opcode", "")
                limit = _WAIT_LIMITS.get(op, 1)
                if op in _WAIT_SKIP or len(waits) <= limit:
                    out.append(inst)
                    continue
                keep = waits[-limit:]
                for j, w in enumerate(waits[:-limit]):
                    n_fix += 1
                    out.append(
                        {
                            "debug": inst.get("debug", 0),
                            "engine": inst["engine"],
                            "ins": [],
                            "name": f"{inst['name']}-wfx{j}",
                            "opcode": "EventSemaphore",
                            "outs": [],
                            "sync_info": {"on_update": [], "on_wait": [w]},
                        }
                    )
                si["on_wait"] = keep
                inst["sync_info"] = si
                out.append(inst)
            blk["instructions"] = out
    return orjson.dumps(d)


def _install_wait_legalizer():
    from concourse import bass2jax

    if getattr(bass2jax, "_wfx_installed", False):
        return
    orig = bass2jax.compile_bir_kernel

    def patched(ant_bir_str, compile_dir, **kw):
        return orig(_legalize_waits(ant_bir_str), compile_dir, **kw)

    bass2jax.compile_bir_kernel = patched
    bass2jax._wfx_installed = True


def _route(x, w_gate, top_k):
    logits = x.astype(np.float64) @ w_gate.T.astype(np.float64)
    logits = 30.0 * np.tanh(logits / 30.0)
    m = logits.max(axis=-1, keepdims=True)
    p = np.exp(logits - m)
    p /= p.sum(axis=-1, keepdims=True)
    order = np.argsort(-p, axis=-1, kind="stable")[:, :top_k]
    combine = np.zeros((x.shape[0], w_gate.shape[0]), dtype=np.float64)
    np.put_along_axis(
        combine, order, np.take_along_axis(p, order, axis=-1), axis=-1
    )
    return combine.astype(np.float32)


def _prep_core(x, w1e, w3e, w2e, combine_e, ix, cap):
    n = len(ix)
    xe = np.zeros((cap, D), dtype=np.float32)
    xe[:n] = x[ix]
    # xc[p, dt, t] = xe[t, dt*128+p]
    xc = np.ascontiguousarray(xe.reshape(cap, ND, P).transpose(2, 1, 0))
    # w13c[it, p, j, dt, m] = wj[it*128+m, dt*128+p]
    w1t = w1e.reshape(NI, P, ND, P).transpose(0, 3, 2, 1)
    w3t = w3e.reshape(NI, P, ND, P).transpose(0, 3, 2, 1)
    w13c = np.ascontiguousarray(np.stack([w1t, w3t], axis=2))
    # w2c[p, it, d] = w2[d, it*128+p]
    w2c = np.ascontiguousarray(w2e.reshape(D, NI, P).transpose(2, 1, 0))
    gw = np.zeros((cap,), dtype=np.float32)
    gw[:n] = combine_e[ix]
    tt = cap // P
    gwc = np.ascontiguousarray(gw.reshape(tt, P).T)
    return {
        "xc": _to_bf16(xc),
        "w13c": _to_bf16(w13c),
        "w2c": _to_bf16(w2c),
        "gwc": gwc,
    }


def kernel(x, w_gate, w1, w3, w2, top_k):
    from concourse.bass_utils import run_bass_kernel_spmd

    _install_wait_legalizer()
    x = np.asarray(x)
    w_gate = np.asarray(w_gate)
    w1 = np.asarray(w1)
    w3 = np.asarray(w3)
    w2 = np.asarray(w2)
    k = int(top_k)

    combine = _route(x, w_gate, k)  # [T, E] fp32, zeros off top-k

    idxs = [np.nonzero(combine[:, e])[0] for e in range(E)]
    cap = C
    maxc = max(len(ix) for ix in idxs)
    if maxc > cap:
        cap = ((maxc + P - 1) // P) * P

    if cap not in _cache:
        _cache[cap] = _build(cap)
    nc = _cache[cap]

    in_maps = [
        _prep_core(x, w1[e], w3[e], w2[e], combine[:, e], idxs[e], cap)
        for e in range(E)
    ]

    res = run_bass_kernel_spmd(nc, in_maps, list(range(E)))

    out = np.zeros((T, D), dtype=np.float32)
    for e in range(E):
        ix = idxs[e]
        ye = np.asarray(res.results[e]["y"], dtype=np.float32)
        out[ix] += ye.reshape(cap, D)[: len(ix)]
    return out


def _to_bf16(a):
    import ml_dtypes

    return np.ascontiguousarray(a).astype(ml_dtypes.bfloat16)
